# revision 7
# baseline (speedup 1.0000x reference)
"""Trainium2 Bass kernel for the LocalConnectivity diamond-ring stencil.

out[b, x, y] = sum_{1<=|dx|+|dy|<=5} w[|dx|+|dy|-1] * in[b, (x+dx)%512, (y+dy)%512]

Strategy (v3)
-------------
Data-parallel over batch: 64 samples -> 8 cores x 8 samples. Per sample the
512x512 grid is processed in 5 UNIFORM row-tiles of 103 output rows (the 5th
tile computes 3 extra wrapped rows that are simply not written back), so all
55 matmuls per sample share identical shapes and the same 11 banded weight
matrices.

The 60-tap stencil runs on the TensorEngine as 11 PSUM-accumulating matmuls
per tile, one per horizontal shift dy in [-5, 5]:

  psum[p, f] += sum_c  WB_dy[c, p] * X[c, f + dy_idx]

where X is the input tile with 5 halo rows on each side (contraction dim =
113 partitions) and 5 circular halo columns on each side, and WB_dy is the
banded Toeplitz matrix holding the vertical taps of kernel column dy.

All-fp16 into the PE (walrus rejects mixed 16/32-bit matmul operands): the
input is cast f32->fp16 inside the gpsimd software-DGE DMA (free), weights
are fp16. Total error ~3e-4 rel absmax, far under tolerance.

GpSimd software-DGE instructions cost ~1us fixed each, so DMAs are merged
across samples: one instruction per tile-plane covering a whole sample
group (DMA APs allow at most partition + 2 free dims, so the group/batch
dim is the mergeable one). Input groups are staggered (2 samples, then 6)
so the first sample's matmuls start after only ~2 samples of input wire
time. Output DMAs cover 4-sample groups. PSUM eviction is split between
ScalarE and VectorE.
"""

import numpy as np

import concourse.bass as bass
import concourse.bacc as bacc
import concourse.mybir as mybir
from concourse import tile
from concourse.bass_utils import run_bass_kernel_spmd

B, H, W = 64, 512, 512
NCORES = 8
BPC = B // NCORES  # samples per core
MAXD = 5
HALO = MAXD
DYS = 2 * MAXD + 1  # 11 horizontal shifts
TR = 103  # output rows per tile (uniform; tile 4 wraps, 3 rows discarded)
NT = 5
CTR = TR + 2 * HALO  # 113 contraction rows
XW = W + 2 * HALO  # 522
HW = H * W

IN_GROUPS = [(0, 2), (2, 6)]  # (b0, size): small first group hides startup
OUT_GROUPS = [(0, 4), (4, 4)]

F16 = mybir.dt.float16


def _build_band_weights(dw: np.ndarray) -> np.ndarray:
    """[128, 11*128]: WB[c, j*128 + p] = K(c-p-5, j-5)."""
    wb = np.zeros((128, DYS, 128), dtype=np.float32)
    p = np.arange(128)
    for j in range(DYS):
        dy = j - MAXD
        for dx in range(-MAXD, MAXD + 1):
            d = abs(dx) + abs(dy)
            if 1 <= d <= MAXD:
                c = p + dx + HALO
                valid = (c >= 0) & (c < 128)
                wb[c[valid], j, p[valid]] = dw[d - 1]
    return np.ascontiguousarray(wb.reshape(128, DYS * 128).astype(np.float16))


_CACHED_NC = None


def _custom_ap(base_ap, dims, extra_offset_elems=0):
    """Build a strided AP: dims = [(stride_elems, size), ...]."""
    s = base_ap.copy()
    s.ap.clear()
    s.ap.extend(dims)
    s.offset = s.offset + extra_offset_elems
    return s


def _load_group(nc, x, xt, b0, gsz):
    """Fill xt[p, b, t, 5+y] = x[b0+b, (103*t - 5 + p) % 512, y] (cast to fp16).

    One gpsimd DMA per rectangular (t, partition-range) region, merged
    across the group's samples (the b free dim).
    """

    def src(prows, row0):
        return _custom_ap(x[b0], [(W, prows), (HW, gsz), (1, W)],
                          extra_offset_elems=row0 * W)

    # bodies: t=0..3 p=5..112 -> rows 103t..103t+107
    for t in range(4):
        nc.gpsimd.dma_start(xt[5:113, :, t, HALO : HALO + W], src(108, TR * t))
    # t=4 body: p=5..104 -> rows 412..511
    nc.gpsimd.dma_start(xt[5:105, :, 4, HALO : HALO + W], src(100, 4 * TR))
    # t=0 wrap-top: p=0..4 -> rows 507..511
    nc.gpsimd.dma_start(xt[0:5, :, 0, HALO : HALO + W], src(5, H - HALO))
    # halo tops t=1..4: p=0..4 -> rows 103t-5..103t-1
    for t in range(1, 5):
        nc.gpsimd.dma_start(xt[0:5, :, t, HALO : HALO + W], src(5, TR * t - HALO))
    # t=4 wrap-bottom: p=105..112 -> rows 0..7
    nc.gpsimd.dma_start(xt[105:113, :, 4, HALO : HALO + W], src(8, 0))

    # circular column halos, split per tile-plane so early planes unblock
    # matmuls without waiting for the whole group
    for t in range(NT):
        nc.scalar.copy(xt[0:113, :, t, 0:HALO], xt[0:113, :, t, W : W + HALO])
        nc.scalar.copy(
            xt[0:113, :, t, HALO + W :], xt[0:113, :, t, HALO : 2 * HALO]
        )


def _build_program():
    f32 = mybir.dt.float32

    nc = bacc.Bacc(None, target_bir_lowering=False)
    x = nc.dram_tensor("x", [BPC, H, W], f32, kind="ExternalInput")
    wb = nc.dram_tensor("wb", [128, DYS * 128], F16, kind="ExternalInput")
    y = nc.dram_tensor("y", [BPC, H, W], f32, kind="ExternalOutput")

    with tile.TileContext(nc) as tc:
        with (
            tc.tile_pool(name="wpool", bufs=1) as wpool,
            tc.tile_pool(name="xpool_a", bufs=1) as xpool_a,
            tc.tile_pool(name="xpool_b", bufs=1) as xpool_b,
            tc.tile_pool(name="opool", bufs=2) as opool,
            tc.tile_pool(name="pspool", bufs=8, space=bass.MemorySpace.PSUM) as pspool,
        ):
            wtile = wpool.tile([128, DYS * 128], F16, tag="wt")
            nc.gpsimd.dma_start(wtile[:], wb[:])

            xts = {}
            (a0, asz), (bb0, bsz) = IN_GROUPS
            xt_a = xpool_a.tile([128, asz, NT, XW], F16, tag="xta")
            _load_group(nc, x, xt_a, a0, asz)
            xts[0] = (xt_a, a0)
            xt_b = xpool_b.tile([128, bsz, NT, XW], F16, tag="xtb")
            _load_group(nc, x, xt_b, bb0, bsz)
            xts[1] = (xt_b, bb0)

            for og, (ob0, osz) in enumerate(OUT_GROUPS):
                otb = opool.tile([128, osz, NT, W], f32, tag="otb")
                for bi in range(osz):
                    b = ob0 + bi
                    gi = 0 if b < IN_GROUPS[1][0] else 1
                    xt, gb0 = xts[gi]
                    bq = b - gb0
                    # ---- 55 matmuls: dy-outer, stationary reused over tiles
                    pts = []
                    for t in range(NT):
                        pt = pspool.tile([128, W], f32, tag="pt")
                        pts.append(pt)
                    for j in range(DYS):
                        lhsT = wtile[0:CTR, j * 128 : j * 128 + TR]
                        for t in range(NT):
                            nc.tensor.matmul(
                                pts[t][0:TR, :],
                                lhsT,
                                xt[0:CTR, bq, t, j : j + W],
                                start=(j == 0),
                                stop=(j == DYS - 1),
                            )
                    # ---- PSUM eviction split Scalar/Vector ----
                    for t in range(NT):
                        if t >= 2:
                            nc.vector.tensor_copy(
                                otb[0:TR, bi, t, :], pts[t][0:TR, :]
                            )
                        else:
                            nc.scalar.copy(otb[0:TR, bi, t, :], pts[t][0:TR, :])

                # ---- merged output DMAs for the group: per tile-plane ----
                for t in range(4):
                    dst = _custom_ap(
                        y[ob0], [(W, TR), (HW, osz), (1, W)],
                        extra_offset_elems=TR * t * W,
                    )
                    nc.gpsimd.dma_start(dst, otb[0:TR, :, t, :])
                dst4 = _custom_ap(
                    y[ob0], [(W, 100), (HW, osz), (1, W)],
                    extra_offset_elems=4 * TR * W,
                )
                nc.gpsimd.dma_start(dst4, otb[0:100, :, 4, :])
    nc.compile()
    return nc


def _get_program():
    global _CACHED_NC
    if _CACHED_NC is None:
        _CACHED_NC = _build_program()
    return _CACHED_NC


def _run(grid_spikes, distance_weights, trace=False):
    grid_spikes = np.ascontiguousarray(np.asarray(grid_spikes, dtype=np.float32))
    distance_weights = np.asarray(distance_weights, dtype=np.float32)
    assert grid_spikes.shape == (B, H, W), grid_spikes.shape
    wb_np = _build_band_weights(distance_weights)

    nc = _get_program()
    in_maps = [
        {
            "x": np.ascontiguousarray(grid_spikes[i * BPC : (i + 1) * BPC]),
            "wb": wb_np,
        }
        for i in range(NCORES)
    ]
    res = run_bass_kernel_spmd(nc, in_maps, list(range(NCORES)), trace=trace)
    out = np.concatenate([res.results[i]["y"] for i in range(NCORES)], axis=0)
    return out.astype(np.float32, copy=False), res


def kernel(grid_spikes, distance_weights):
    out, _ = _run(grid_spikes, distance_weights, trace=False)
    return out


def kernel_traced(grid_spikes, distance_weights):
    out, res = _run(grid_spikes, distance_weights, trace=True)
    return out, res


# revision 9
# speedup vs baseline: 1.3093x; 1.3093x over previous
"""Trainium2 Bass kernel for the LocalConnectivity diamond-ring stencil.

out[b, x, y] = sum_{1<=|dx|+|dy|<=5} w[|dx|+|dy|-1] * in[b, (x+dx)%512, (y+dy)%512]

Strategy (v4)
-------------
Data-parallel over batch: 64 samples -> 8 cores x 8 samples. Per sample the
512x512 grid is processed in 5 UNIFORM row-tiles of 103 output rows (the 5th
tile computes 3 extra wrapped rows that are simply not written back), so all
55 matmuls per sample share identical shapes and the same 11 banded weight
matrices (fp16; input cast f32->fp16 inside the gpsimd software-DGE DMA).

The 60-tap stencil runs on the TensorEngine as 11 PSUM-accumulating matmuls
per tile, one per horizontal shift dy in [-5, 5], dy-outer so the stationary
matrix is reused across tiles; measured back-to-back at 216 ns (warm HAM).

Measured pipeline facts driving the layout:
 - GpSimd software-DGE instructions cost ~1-2 us each (fixed + per-row),
   so INPUT DMAs are merged: one instruction per tile-plane covering a
   sample group (DMA APs allow partition + 2 free dims; the batch dim is
   merged). Groups are staggered (1, 3, 4 samples) so sample 0's matmuls
   start after ~10 us.
 - HBM READS tolerate the resulting scattered access, but scattered WRITES
   crawl (~50 GB/s vs 300+), so OUTPUT DMAs are per-(sample, tile) fully
   contiguous 211 KB transfers. They are spread across the sync/scalar/
   vector hardware-DGE queues (~18 GB/s each, no GpSimd cost) for tiles
   0/2/4 and gpsimd software-DGE (16-engine fan-out) for tiles 1/3.
 - PSUM eviction is split ScalarE (t=0,1) / VectorE (t=2,3,4).
"""

import numpy as np

import concourse.bass as bass
import concourse.bacc as bacc
import concourse.mybir as mybir
from concourse import tile
from concourse.bass_utils import run_bass_kernel_spmd

B, H, W = 64, 512, 512
NCORES = 8
BPC = B // NCORES  # samples per core
MAXD = 5
HALO = MAXD
DYS = 2 * MAXD + 1  # 11 horizontal shifts
TR = 103  # output rows per tile (uniform; tile 4 wraps, 3 rows discarded)
NT = 5
CTR = TR + 2 * HALO  # 113 contraction rows
XW = W + 2 * HALO  # 522
HW = H * W

IN_GROUPS = [(0, 1), (1, 3), (4, 4)]  # (b0, size); first small to hide startup

F16 = mybir.dt.float16


def _build_band_weights(dw: np.ndarray) -> np.ndarray:
    """[128, 11*128]: WB[c, j*128 + p] = K(c-p-5, j-5)."""
    wb = np.zeros((128, DYS, 128), dtype=np.float32)
    p = np.arange(128)
    for j in range(DYS):
        dy = j - MAXD
        for dx in range(-MAXD, MAXD + 1):
            d = abs(dx) + abs(dy)
            if 1 <= d <= MAXD:
                c = p + dx + HALO
                valid = (c >= 0) & (c < 128)
                wb[c[valid], j, p[valid]] = dw[d - 1]
    return np.ascontiguousarray(wb.reshape(128, DYS * 128).astype(np.float16))


_CACHED_NC = None


def _custom_ap(base_ap, dims, extra_offset_elems=0):
    """Build a strided AP: dims = [(stride_elems, size), ...]."""
    s = base_ap.copy()
    s.ap.clear()
    s.ap.extend(dims)
    s.offset = s.offset + extra_offset_elems
    return s


def _load_group(nc, x, xt, b0, gsz):
    """Fill xt[p, b, t, 5+y] = x[b0+b, (103*t - 5 + p) % 512, y] (cast fp16).

    gsz==1 merges the tile dim (5 instructions); gsz>1 merges the batch dim
    (11 instructions, one per rectangular tile-plane region).
    """
    if gsz == 1:
        def src1(prows, row0):
            return _custom_ap(x[b0], [(W, prows), (1, W)],
                              extra_offset_elems=row0 * W)
        # bodies t=0..3 (t in a free dim), rows 103t + p - 5, p=5..112
        src_body = _custom_ap(x[b0], [(W, 108), (TR * W, 4), (1, W)])
        nc.gpsimd.dma_start(xt[5:113, 0, 0:4, HALO : HALO + W], src_body)
        nc.gpsimd.dma_start(xt[5:105, 0, 4, HALO : HALO + W], src1(100, 4 * TR))
        nc.gpsimd.dma_start(xt[0:5, 0, 0, HALO : HALO + W], src1(5, H - HALO))
        src_tops = _custom_ap(x[b0], [(W, 5), (TR * W, 4), (1, W)],
                              extra_offset_elems=98 * W)
        nc.gpsimd.dma_start(xt[0:5, 0, 1:5, HALO : HALO + W], src_tops)
        nc.gpsimd.dma_start(xt[105:113, 0, 4, HALO : HALO + W], src1(8, 0))
    else:
        def src(prows, row0):
            return _custom_ap(x[b0], [(W, prows), (HW, gsz), (1, W)],
                              extra_offset_elems=row0 * W)
        for t in range(4):
            nc.gpsimd.dma_start(xt[5:113, :, t, HALO : HALO + W],
                                src(108, TR * t))
        nc.gpsimd.dma_start(xt[5:105, :, 4, HALO : HALO + W], src(100, 4 * TR))
        nc.gpsimd.dma_start(xt[0:5, :, 0, HALO : HALO + W], src(5, H - HALO))
        for t in range(1, 5):
            nc.gpsimd.dma_start(xt[0:5, :, t, HALO : HALO + W],
                                src(5, TR * t - HALO))
        nc.gpsimd.dma_start(xt[105:113, :, 4, HALO : HALO + W], src(8, 0))

    # circular column halos, per tile-plane
    for t in range(NT):
        nc.scalar.copy(xt[0:113, :, t, 0:HALO], xt[0:113, :, t, W : W + HALO])
        nc.scalar.copy(
            xt[0:113, :, t, HALO + W :], xt[0:113, :, t, HALO : 2 * HALO]
        )


def _build_program():
    f32 = mybir.dt.float32

    nc = bacc.Bacc(None, target_bir_lowering=False)
    x = nc.dram_tensor("x", [BPC, H, W], f32, kind="ExternalInput")
    wb = nc.dram_tensor("wb", [128, DYS * 128], F16, kind="ExternalInput")
    y = nc.dram_tensor("y", [BPC, H, W], f32, kind="ExternalOutput")

    with tile.TileContext(nc) as tc:
        with (
            tc.tile_pool(name="wpool", bufs=1) as wpool,
            tc.tile_pool(name="xpool_a", bufs=1) as xpool_a,
            tc.tile_pool(name="xpool_b", bufs=1) as xpool_b,
            tc.tile_pool(name="xpool_c", bufs=1) as xpool_c,
            tc.tile_pool(name="opool", bufs=3) as opool,
            tc.tile_pool(name="pspool", bufs=8, space=bass.MemorySpace.PSUM) as pspool,
        ):
            wtile = wpool.tile([128, DYS * 128], F16, tag="wt")
            nc.gpsimd.dma_start(wtile[:], wb[:])

            sample_xt = {}
            for pool, (b0, gsz) in zip(
                (xpool_a, xpool_b, xpool_c), IN_GROUPS
            ):
                xt = pool.tile([128, gsz, NT, XW], F16, tag=f"xt{b0}")
                _load_group(nc, x, xt, b0, gsz)
                for bi in range(gsz):
                    sample_xt[b0 + bi] = (xt, bi)

            for b in range(BPC):
                xt, bq = sample_xt[b]
                # ---- 55 matmuls: dy-outer, stationary reused over tiles ----
                pts = []
                for t in range(NT):
                    pt = pspool.tile([128, W], f32, tag="pt")
                    pts.append(pt)
                for j in range(DYS):
                    lhsT = wtile[0:CTR, j * 128 : j * 128 + TR]
                    for t in range(NT):
                        nc.tensor.matmul(
                            pts[t][0:TR, :],
                            lhsT,
                            xt[0:CTR, bq, t, j : j + W],
                            start=(j == 0),
                            stop=(j == DYS - 1),
                        )

                # ---- PSUM eviction split Scalar/Vector ----
                otb = opool.tile([128, NT, W], f32, tag="otb")
                for t in range(NT):
                    if t >= 2:
                        nc.vector.tensor_copy(otb[0:TR, t, :], pts[t][0:TR, :])
                    else:
                        nc.scalar.copy(otb[0:TR, t, :], pts[t][0:TR, :])

                # ---- per-(sample, tile) contiguous output DMAs, spread
                # across HWDGE queues (sync/scalar/vector) + gpsimd ----
                nc.sync.dma_start(y[b, 0:TR, :], otb[0:TR, 0, :])
                nc.gpsimd.dma_start(y[b, TR : 2 * TR, :], otb[0:TR, 1, :])
                nc.scalar.dma_start(y[b, 2 * TR : 3 * TR, :], otb[0:TR, 2, :])
                nc.gpsimd.dma_start(y[b, 3 * TR : 4 * TR, :], otb[0:TR, 3, :])
                nc.gpsimd.dma_start(y[b, 4 * TR : H, :], otb[0:100, 4, :])
    nc.compile()
    return nc


def _get_program():
    global _CACHED_NC
    if _CACHED_NC is None:
        _CACHED_NC = _build_program()
    return _CACHED_NC


def _run(grid_spikes, distance_weights, trace=False):
    grid_spikes = np.ascontiguousarray(np.asarray(grid_spikes, dtype=np.float32))
    distance_weights = np.asarray(distance_weights, dtype=np.float32)
    assert grid_spikes.shape == (B, H, W), grid_spikes.shape
    wb_np = _build_band_weights(distance_weights)

    nc = _get_program()
    in_maps = [
        {
            "x": np.ascontiguousarray(grid_spikes[i * BPC : (i + 1) * BPC]),
            "wb": wb_np,
        }
        for i in range(NCORES)
    ]
    res = run_bass_kernel_spmd(nc, in_maps, list(range(NCORES)), trace=trace)
    out = np.concatenate([res.results[i]["y"] for i in range(NCORES)], axis=0)
    return out.astype(np.float32, copy=False), res


def kernel(grid_spikes, distance_weights):
    out, _ = _run(grid_spikes, distance_weights, trace=False)
    return out


def kernel_traced(grid_spikes, distance_weights):
    out, res = _run(grid_spikes, distance_weights, trace=True)
    return out, res


# revision 11
# speedup vs baseline: 2.0413x; 1.5591x over previous
"""Trainium2 Bass kernel for the LocalConnectivity diamond-ring stencil.

out[b, x, y] = sum_{1<=|dx|+|dy|<=5} w[|dx|+|dy|-1] * in[b, (x+dx)%512, (y+dy)%512]

Strategy (v6)
-------------
Data-parallel over batch: 64 samples -> 8 cores x 8 samples. Per sample the
512x512 grid is processed in 5 uniform row-tiles of 103 output rows (the 5th
tile computes 3 extra wrapped rows that the host drops), so all 55 matmuls
per sample share identical shapes and the same 11 banded fp16 weight
matrices (input is cast f32->fp16 inside the gpsimd software-DGE DMA).

The 60-tap stencil runs on the TensorEngine as 11 PSUM-accumulating matmuls
per tile, one per horizontal shift dy in [-5, 5], dy-outer so the stationary
matrix is reused across tiles; measured back-to-back at 216 ns/matmul (warm).

THE central hardware fact (microbenchmarked): gpsimd software-DGE DMAs
whose SBUF side spans all 128 partitions run at 300-470 GB/s; any partial
partition range (103/113 rows) takes a degraded ~43 GB/s path on 2 SDMA
engines. So every bulk transfer here is a full-128-partition DMA:

 - INPUT is padded on the host to [8, 540, 512] per core (5 wrap rows in
   front, 23 behind), so each tile-plane loads rows 103t..103t+127 as one
   [128, gsz, 512] DMA with no wrap/halo fixup DMAs at all. 10 input DMAs
   per core, in two sample-groups (2 then 6) to hide startup.
 - OUTPUT goes to a padded y [8, 5*128, 512]: one [128, 5, 512] DMA per
   sample (tile rows 103..127 are garbage the host slices off).
 - Circular COLUMN halos are on-chip ScalarE copies; PSUM eviction is
   split ScalarE (t=0,1) / VectorE (t=2,3,4).
"""

import numpy as np

import concourse.bass as bass
import concourse.bacc as bacc
import concourse.mybir as mybir
from concourse import tile
from concourse.bass_utils import run_bass_kernel_spmd

B, H, W = 64, 512, 512
NCORES = 8
BPC = B // NCORES  # samples per core
MAXD = 5
HALO = MAXD
DYS = 2 * MAXD + 1  # 11 horizontal shifts
TR = 103  # output rows per tile
NT = 5
CTR = TR + 2 * HALO  # 113 contraction rows
XW = W + 2 * HALO  # 522
HPAD = HALO + H + 23  # 540 padded input rows per sample
HOUT = NT * 128  # 640 padded output rows per sample

IN_GROUPS = [(0, 2), (2, 6)]  # (b0, size); small first group hides startup

F16 = mybir.dt.float16


def _build_band_weights(dw: np.ndarray) -> np.ndarray:
    """[128, 11*128]: WB[c, j*128 + p] = K(c-p-5, j-5)."""
    wb = np.zeros((128, DYS, 128), dtype=np.float32)
    p = np.arange(128)
    for j in range(DYS):
        dy = j - MAXD
        for dx in range(-MAXD, MAXD + 1):
            d = abs(dx) + abs(dy)
            if 1 <= d <= MAXD:
                c = p + dx + HALO
                valid = (c >= 0) & (c < 128)
                wb[c[valid], j, p[valid]] = dw[d - 1]
    return np.ascontiguousarray(wb.reshape(128, DYS * 128).astype(np.float16))


_CACHED_NC = None


def _custom_ap(base_ap, dims, extra_offset_elems=0):
    """Build a strided AP: dims = [(stride_elems, size), ...]."""
    s = base_ap.copy()
    s.ap.clear()
    s.ap.extend(dims)
    s.offset = s.offset + extra_offset_elems
    return s


def _build_program():
    f32 = mybir.dt.float32

    nc = bacc.Bacc(None, target_bir_lowering=False)
    x = nc.dram_tensor("x", [BPC, HPAD, W], f32, kind="ExternalInput")
    wb = nc.dram_tensor("wb", [128, DYS * 128], F16, kind="ExternalInput")
    y = nc.dram_tensor("y", [BPC, HOUT, W], f32, kind="ExternalOutput")

    with tile.TileContext(nc) as tc:
        with (
            tc.tile_pool(name="wpool", bufs=1) as wpool,
            tc.tile_pool(name="xpool_a", bufs=1) as xpool_a,
            tc.tile_pool(name="xpool_b", bufs=1) as xpool_b,
            tc.tile_pool(name="opool", bufs=3) as opool,
            tc.tile_pool(name="pspool", bufs=8, space=bass.MemorySpace.PSUM) as pspool,
        ):
            wtile = wpool.tile([128, DYS * 128], F16, tag="wt")
            nc.gpsimd.dma_start(wtile[:], wb[:])

            sample_xt = {}
            for pool, (b0, gsz) in zip((xpool_a, xpool_b), IN_GROUPS):
                # xt[p, b, t, 5+y] = xpad[b0+b, 103t + p, y]
                #                  = x_orig[b0+b, (103t - 5 + p) % 512, y]
                xt = pool.tile([128, gsz, NT, XW], F16, tag=f"xt{b0}")
                for t in range(NT):
                    src = _custom_ap(
                        x[b0], [(W, 128), (HPAD * W, gsz), (1, W)],
                        extra_offset_elems=TR * t * W,
                    )
                    nc.gpsimd.dma_start(xt[:, :, t, HALO : HALO + W], src)
                    # circular column halos for this plane
                    nc.scalar.copy(
                        xt[0:CTR, :, t, 0:HALO], xt[0:CTR, :, t, W : W + HALO]
                    )
                    nc.scalar.copy(
                        xt[0:CTR, :, t, HALO + W :],
                        xt[0:CTR, :, t, HALO : 2 * HALO],
                    )
                for bi in range(gsz):
                    sample_xt[b0 + bi] = (xt, bi)

            for b in range(BPC):
                xt, bq = sample_xt[b]
                # ---- 55 matmuls: dy-outer, stationary reused over tiles ----
                pts = []
                for t in range(NT):
                    pt = pspool.tile([128, W], f32, tag="pt")
                    pts.append(pt)
                for j in range(DYS):
                    lhsT = wtile[0:CTR, j * 128 : j * 128 + TR]
                    for t in range(NT):
                        nc.tensor.matmul(
                            pts[t][0:TR, :],
                            lhsT,
                            xt[0:CTR, bq, t, j : j + W],
                            start=(j == 0),
                            stop=(j == DYS - 1),
                        )

                # ---- PSUM eviction split Scalar/Vector ----
                otb = opool.tile([128, NT, W], f32, tag="otb")
                for t in range(NT):
                    if t >= 2:
                        nc.vector.tensor_copy(otb[0:TR, t, :], pts[t][0:TR, :])
                    else:
                        nc.scalar.copy(otb[0:TR, t, :], pts[t][0:TR, :])

                # ---- one full-128-partition output DMA per sample ----
                dst = _custom_ap(
                    y[b], [(W, 128), (128 * W, NT), (1, W)]
                )
                nc.gpsimd.dma_start(dst, otb[:, :, :])
    nc.compile()
    return nc


def _get_program():
    global _CACHED_NC
    if _CACHED_NC is None:
        _CACHED_NC = _build_program()
    return _CACHED_NC


def _run(grid_spikes, distance_weights, trace=False):
    grid_spikes = np.ascontiguousarray(np.asarray(grid_spikes, dtype=np.float32))
    distance_weights = np.asarray(distance_weights, dtype=np.float32)
    assert grid_spikes.shape == (B, H, W), grid_spikes.shape
    wb_np = _build_band_weights(distance_weights)

    # pad each sample: 5 wrap rows in front (507..511), 23 behind (0..22)
    xpad = np.concatenate(
        [grid_spikes[:, H - HALO :, :], grid_spikes, grid_spikes[:, :23, :]],
        axis=1,
    )  # [B, 540, W]
    assert xpad.shape[1] == HPAD

    nc = _get_program()
    in_maps = [
        {
            "x": np.ascontiguousarray(xpad[i * BPC : (i + 1) * BPC]),
            "wb": wb_np,
        }
        for i in range(NCORES)
    ]
    res = run_bass_kernel_spmd(nc, in_maps, list(range(NCORES)), trace=trace)
    ypad = np.concatenate(
        [res.results[i]["y"] for i in range(NCORES)], axis=0
    )  # [B, 640, W]
    # unpack: row 103t + p lives at padded row 128t + p (p < 103); the last
    # tile's rows 512..514 are circular duplicates the slice drops.
    out = (
        ypad.reshape(B, NT, 128, W)[:, :, :TR, :]
        .reshape(B, NT * TR, W)[:, :H, :]
    )
    return np.ascontiguousarray(out, dtype=np.float32), res


def kernel(grid_spikes, distance_weights):
    out, _ = _run(grid_spikes, distance_weights, trace=False)
    return out


def kernel_traced(grid_spikes, distance_weights):
    out, res = _run(grid_spikes, distance_weights, trace=True)
    return out, res


# revision 12
# speedup vs baseline: 2.0642x; 1.0112x over previous
"""Trainium2 Bass kernel for the LocalConnectivity diamond-ring stencil.

out[b, x, y] = sum_{1<=|dx|+|dy|<=5} w[|dx|+|dy|-1] * in[b, (x+dx)%512, (y+dy)%512]

Strategy (v6)
-------------
Data-parallel over batch: 64 samples -> 8 cores x 8 samples. Per sample the
512x512 grid is processed in 5 uniform row-tiles of 103 output rows (the 5th
tile computes 3 extra wrapped rows that the host drops), so all 55 matmuls
per sample share identical shapes and the same 11 banded fp16 weight
matrices (input is cast f32->fp16 inside the gpsimd software-DGE DMA).

The 60-tap stencil runs on the TensorEngine as 11 PSUM-accumulating matmuls
per tile, one per horizontal shift dy in [-5, 5], dy-outer so the stationary
matrix is reused across tiles; measured back-to-back at 216 ns/matmul (warm).

THE central hardware fact (microbenchmarked): gpsimd software-DGE DMAs
whose SBUF side spans all 128 partitions run at 300-470 GB/s; any partial
partition range (103/113 rows) takes a degraded ~43 GB/s path on 2 SDMA
engines. So every bulk transfer here is a full-128-partition DMA:

 - INPUT is padded on the host to [8, 540, 512] per core (5 wrap rows in
   front, 23 behind), so each tile-plane loads rows 103t..103t+127 as one
   [128, gsz, 512] DMA with no wrap/halo fixup DMAs at all. 10 input DMAs
   per core, in two sample-groups (2 then 6) to hide startup.
 - OUTPUT goes to a padded y [8, 5*128, 512]: one [128, 5, 512] DMA per
   sample (tile rows 103..127 are garbage the host slices off).
 - Circular COLUMN halos are on-chip ScalarE copies; PSUM eviction is
   split ScalarE (t=0,1) / VectorE (t=2,3,4).
"""

import numpy as np

import concourse.bass as bass
import concourse.bacc as bacc
import concourse.mybir as mybir
from concourse import tile
from concourse.bass_utils import run_bass_kernel_spmd

B, H, W = 64, 512, 512
NCORES = 8
BPC = B // NCORES  # samples per core
MAXD = 5
HALO = MAXD
DYS = 2 * MAXD + 1  # 11 horizontal shifts
TR = 103  # output rows per tile
NT = 5
CTR = TR + 2 * HALO  # 113 contraction rows
XW = W + 2 * HALO  # 522
HPAD = HALO + H + 23  # 540 padded input rows per sample
HOUT = NT * 128  # 640 padded output rows per sample

IN_GROUPS = [(0, 2), (2, 3), (5, 3)]  # (b0, size); staggered to hide wire time

F16 = mybir.dt.float16


def _build_band_weights(dw: np.ndarray) -> np.ndarray:
    """[128, 11*128]: WB[c, j*128 + p] = K(c-p-5, j-5)."""
    wb = np.zeros((128, DYS, 128), dtype=np.float32)
    p = np.arange(128)
    for j in range(DYS):
        dy = j - MAXD
        for dx in range(-MAXD, MAXD + 1):
            d = abs(dx) + abs(dy)
            if 1 <= d <= MAXD:
                c = p + dx + HALO
                valid = (c >= 0) & (c < 128)
                wb[c[valid], j, p[valid]] = dw[d - 1]
    return np.ascontiguousarray(wb.reshape(128, DYS * 128).astype(np.float16))


_CACHED_NC = None


def _custom_ap(base_ap, dims, extra_offset_elems=0):
    """Build a strided AP: dims = [(stride_elems, size), ...]."""
    s = base_ap.copy()
    s.ap.clear()
    s.ap.extend(dims)
    s.offset = s.offset + extra_offset_elems
    return s


def _build_program():
    f32 = mybir.dt.float32

    nc = bacc.Bacc(None, target_bir_lowering=False)
    x = nc.dram_tensor("x", [BPC, HPAD, W], f32, kind="ExternalInput")
    wb = nc.dram_tensor("wb", [128, DYS * 128], F16, kind="ExternalInput")
    y = nc.dram_tensor("y", [BPC, HOUT, W], f32, kind="ExternalOutput")

    with tile.TileContext(nc) as tc:
        with (
            tc.tile_pool(name="wpool", bufs=1) as wpool,
            tc.tile_pool(name="xpool_a", bufs=1) as xpool_a,
            tc.tile_pool(name="xpool_b", bufs=1) as xpool_b,
            tc.tile_pool(name="xpool_c", bufs=1) as xpool_c,
            tc.tile_pool(name="opool", bufs=3) as opool,
            tc.tile_pool(name="pspool", bufs=8, space=bass.MemorySpace.PSUM) as pspool,
        ):
            wtile = wpool.tile([128, DYS * 128], F16, tag="wt")

            sample_xt = {}
            for gi, (pool, (b0, gsz)) in enumerate(
                zip((xpool_a, xpool_b, xpool_c), IN_GROUPS)
            ):
                # xt[p, b, t, 5+y] = xpad[b0+b, 103t + p, y]
                #                  = x_orig[b0+b, (103t - 5 + p) % 512, y]
                xt = pool.tile([128, gsz, NT, XW], F16, tag=f"xt{b0}")
                for t in range(NT):
                    src = _custom_ap(
                        x[b0], [(W, 128), (HPAD * W, gsz), (1, W)],
                        extra_offset_elems=TR * t * W,
                    )
                    nc.gpsimd.dma_start(xt[:, :, t, HALO : HALO + W], src)
                    # circular column halos for this plane
                    nc.scalar.copy(
                        xt[0:CTR, :, t, 0:HALO], xt[0:CTR, :, t, W : W + HALO]
                    )
                    nc.scalar.copy(
                        xt[0:CTR, :, t, HALO + W :],
                        xt[0:CTR, :, t, HALO : 2 * HALO],
                    )
                for bi in range(gsz):
                    sample_xt[b0 + bi] = (xt, bi)
                if gi == 0:
                    # weights land between group-0's planes and everyone
                    # else's; only the first matmul needs them
                    nc.gpsimd.dma_start(wtile[:], wb[:])

            for b in range(BPC):
                xt, bq = sample_xt[b]
                # ---- 55 matmuls: dy-outer, stationary reused over tiles ----
                pts = []
                for t in range(NT):
                    pt = pspool.tile([128, W], f32, tag="pt")
                    pts.append(pt)
                if b == 0:
                    loop = [(j, t) for t in range(NT) for j in range(DYS)]
                else:
                    loop = [(j, t) for j in range(DYS) for t in range(NT)]
                for j, t in loop:
                    nc.tensor.matmul(
                        pts[t][0:TR, :],
                        wtile[0:CTR, j * 128 : j * 128 + TR],
                        xt[0:CTR, bq, t, j : j + W],
                        start=(j == 0),
                        stop=(j == DYS - 1),
                    )

                # ---- PSUM eviction split Scalar/Vector ----
                otb = opool.tile([128, NT, W], f32, tag="otb")
                for t in range(NT):
                    if t >= 2:
                        nc.vector.tensor_copy(otb[0:TR, t, :], pts[t][0:TR, :])
                    else:
                        nc.scalar.copy(otb[0:TR, t, :], pts[t][0:TR, :])

                # ---- one full-128-partition output DMA per sample ----
                dst = _custom_ap(
                    y[b], [(W, 128), (128 * W, NT), (1, W)]
                )
                nc.gpsimd.dma_start(dst, otb[:, :, :])
    nc.compile()
    return nc


def _get_program():
    global _CACHED_NC
    if _CACHED_NC is None:
        _CACHED_NC = _build_program()
    return _CACHED_NC


def _run(grid_spikes, distance_weights, trace=False):
    grid_spikes = np.ascontiguousarray(np.asarray(grid_spikes, dtype=np.float32))
    distance_weights = np.asarray(distance_weights, dtype=np.float32)
    assert grid_spikes.shape == (B, H, W), grid_spikes.shape
    wb_np = _build_band_weights(distance_weights)

    # pad each sample: 5 wrap rows in front (507..511), 23 behind (0..22)
    xpad = np.concatenate(
        [grid_spikes[:, H - HALO :, :], grid_spikes, grid_spikes[:, :23, :]],
        axis=1,
    )  # [B, 540, W]
    assert xpad.shape[1] == HPAD

    nc = _get_program()
    in_maps = [
        {
            "x": np.ascontiguousarray(xpad[i * BPC : (i + 1) * BPC]),
            "wb": wb_np,
        }
        for i in range(NCORES)
    ]
    res = run_bass_kernel_spmd(nc, in_maps, list(range(NCORES)), trace=trace)
    ypad = np.concatenate(
        [res.results[i]["y"] for i in range(NCORES)], axis=0
    )  # [B, 640, W]
    # unpack: row 103t + p lives at padded row 128t + p (p < 103); the last
    # tile's rows 512..514 are circular duplicates the slice drops.
    out = (
        ypad.reshape(B, NT, 128, W)[:, :, :TR, :]
        .reshape(B, NT * TR, W)[:, :H, :]
    )
    return np.ascontiguousarray(out, dtype=np.float32), res


def kernel(grid_spikes, distance_weights):
    out, _ = _run(grid_spikes, distance_weights, trace=False)
    return out


def kernel_traced(grid_spikes, distance_weights):
    out, res = _run(grid_spikes, distance_weights, trace=True)
    return out, res


# revision 14
# speedup vs baseline: 2.2202x; 1.0755x over previous
"""Trainium2 Bass kernel for the LocalConnectivity diamond-ring stencil.

out[b, x, y] = sum_{1<=|dx|+|dy|<=5} w[|dx|+|dy|-1] * in[b, (x+dx)%512, (y+dy)%512]

Strategy (v6)
-------------
Data-parallel over batch: 64 samples -> 8 cores x 8 samples. Per sample the
512x512 grid is processed in 5 uniform row-tiles of 103 output rows (the 5th
tile computes 3 extra wrapped rows that the host drops), so all 55 matmuls
per sample share identical shapes and the same 11 banded fp16 weight
matrices (input is cast f32->fp16 inside the gpsimd software-DGE DMA).

The 60-tap stencil runs on the TensorEngine as 11 PSUM-accumulating matmuls
per tile, one per horizontal shift dy in [-5, 5], dy-outer so the stationary
matrix is reused across tiles; measured back-to-back at 216 ns/matmul (warm).

THE central hardware fact (microbenchmarked): gpsimd software-DGE DMAs
whose SBUF side spans all 128 partitions run at 300-470 GB/s; any partial
partition range (103/113 rows) takes a degraded ~43 GB/s path on 2 SDMA
engines. So every bulk transfer here is a full-128-partition DMA:

 - INPUT is padded on the host to [8, 540, 512] per core (5 wrap rows in
   front, 23 behind), so each tile-plane loads rows 103t..103t+127 as one
   [128, gsz, 512] DMA with no wrap/halo fixup DMAs at all. 10 input DMAs
   per core, in two sample-groups (2 then 6) to hide startup.
 - OUTPUT goes to a padded y [8, 5*128, 512]: one [128, 5, 512] DMA per
   sample (tile rows 103..127 are garbage the host slices off).
 - Circular COLUMN halos are on-chip ScalarE copies; PSUM eviction is
   split ScalarE (t=0,1) / VectorE (t=2,3,4).
"""

import numpy as np

import concourse.bass as bass
import concourse.bacc as bacc
import concourse.mybir as mybir
from concourse import tile
from concourse.bass_utils import run_bass_kernel_spmd

B, H, W = 64, 512, 512
NCORES = 8
BPC = B // NCORES  # samples per core
MAXD = 5
HALO = MAXD
DYS = 2 * MAXD + 1  # 11 horizontal shifts
TR = 103  # output rows per tile
NT = 5
CTR = TR + 2 * HALO  # 113 contraction rows
XW = W + 2 * HALO  # 522
HPAD = HALO + H + 23  # 540 padded input rows per sample
HOUT = NT * 128  # 640 padded output rows per sample

IN_GROUPS = [(0, 2), (2, 3), (5, 3)]  # (b0, size); staggered to hide wire time

F16 = mybir.dt.float16


def _build_band_weights(dw: np.ndarray) -> np.ndarray:
    """[128, 11*128]: WB[c, j*128 + p] = K(c-p-5, j-5)."""
    wb = np.zeros((128, DYS, 128), dtype=np.float32)
    p = np.arange(128)
    for j in range(DYS):
        dy = j - MAXD
        for dx in range(-MAXD, MAXD + 1):
            d = abs(dx) + abs(dy)
            if 1 <= d <= MAXD:
                c = p + dx + HALO
                valid = (c >= 0) & (c < 128)
                wb[c[valid], j, p[valid]] = dw[d - 1]
    return np.ascontiguousarray(wb.reshape(128, DYS * 128).astype(np.float16))


_CACHED_NC = None


def _custom_ap(base_ap, dims, extra_offset_elems=0):
    """Build a strided AP: dims = [(stride_elems, size), ...]."""
    s = base_ap.copy()
    s.ap.clear()
    s.ap.extend(dims)
    s.offset = s.offset + extra_offset_elems
    return s


def _build_program():
    f32 = mybir.dt.float32

    nc = bacc.Bacc(None, target_bir_lowering=False)
    x = nc.dram_tensor("x", [BPC, HPAD, W], f32, kind="ExternalInput")
    wb = nc.dram_tensor("wb", [128, DYS * 128], F16, kind="ExternalInput")
    y = nc.dram_tensor("y", [BPC, HOUT, W], f32, kind="ExternalOutput")

    with tile.TileContext(nc) as tc:
        with (
            tc.tile_pool(name="wpool", bufs=1) as wpool,
            tc.tile_pool(name="xpool_a", bufs=1) as xpool_a,
            tc.tile_pool(name="xpool_b", bufs=1) as xpool_b,
            tc.tile_pool(name="xpool_c", bufs=1) as xpool_c,
            tc.tile_pool(name="opool", bufs=3) as opool,
            tc.tile_pool(name="pspool", bufs=8, space=bass.MemorySpace.PSUM) as pspool,
        ):
            wtile = wpool.tile([128, DYS * 128], F16, tag="wt")
            nc.gpsimd.dma_start(wtile[:], wb[:])

            # PE warm-up: ~16 dummy matmuls during the input-load window
            # trip the HAM clock-gate to 8/8 before the real stream starts
            dummy = wpool.tile([128, 640], F16, tag="dummy")
            nc.vector.memset(dummy[:], 0.0)
            wpt = pspool.tile([128, W], mybir.dt.float32, tag="pt")
            for _ in range(16):
                nc.tensor.matmul(wpt[0:TR, :], dummy[0:CTR, 0:TR],
                                 dummy[0:CTR, 64:576], start=True, stop=True)

            sample_xt = {}
            for gi, (pool, (b0, gsz)) in enumerate(
                zip((xpool_a, xpool_b, xpool_c), IN_GROUPS)
            ):
                # xt[p, b, t, 5+y] = xpad[b0+b, 103t + p, y]
                #                  = x_orig[b0+b, (103t - 5 + p) % 512, y]
                xt = pool.tile([128, gsz, NT, XW], F16, tag=f"xt{b0}")
                for t in range(NT):
                    src = _custom_ap(
                        x[b0], [(W, 128), (HPAD * W, gsz), (1, W)],
                        extra_offset_elems=TR * t * W,
                    )
                    nc.gpsimd.dma_start(xt[:, :, t, HALO : HALO + W], src)
                    # circular column halos for this plane
                    nc.scalar.copy(
                        xt[0:CTR, :, t, 0:HALO], xt[0:CTR, :, t, W : W + HALO]
                    )
                    nc.scalar.copy(
                        xt[0:CTR, :, t, HALO + W :],
                        xt[0:CTR, :, t, HALO : 2 * HALO],
                    )
                for bi in range(gsz):
                    sample_xt[b0 + bi] = (xt, bi)

            for b in range(BPC):
                xt, bq = sample_xt[b]
                # ---- 55 matmuls: dy-outer, stationary reused over tiles ----
                pts = []
                for t in range(NT):
                    pt = pspool.tile([128, W], f32, tag="pt")
                    pts.append(pt)
                if b == 0:
                    loop = [(j, t) for t in range(NT) for j in range(DYS)]
                else:
                    loop = [(j, t) for j in range(DYS) for t in range(NT)]
                for j, t in loop:
                    nc.tensor.matmul(
                        pts[t][0:TR, :],
                        wtile[0:CTR, j * 128 : j * 128 + TR],
                        xt[0:CTR, bq, t, j : j + W],
                        start=(j == 0),
                        stop=(j == DYS - 1),
                    )

                # ---- PSUM eviction split Scalar/Vector ----
                otb = opool.tile([128, NT, W], f32, tag="otb")
                for t in range(NT):
                    if t >= 2:
                        nc.vector.tensor_copy(otb[0:TR, t, :], pts[t][0:TR, :])
                    else:
                        nc.scalar.copy(otb[0:TR, t, :], pts[t][0:TR, :])

                # ---- full-128-partition output DMAs ----
                if b == BPC - 1:
                    dst0 = _custom_ap(y[b], [(W, 128), (128 * W, 3), (1, W)])
                    nc.gpsimd.dma_start(dst0, otb[:, 0:3, :])
                    dst1 = _custom_ap(
                        y[b], [(W, 128), (128 * W, 2), (1, W)],
                        extra_offset_elems=3 * 128 * W,
                    )
                    nc.gpsimd.dma_start(dst1, otb[:, 3:5, :])
                else:
                    dst = _custom_ap(y[b], [(W, 128), (128 * W, NT), (1, W)])
                    nc.gpsimd.dma_start(dst, otb[:, :, :])
    nc.compile()
    return nc


def _get_program():
    global _CACHED_NC
    if _CACHED_NC is None:
        _CACHED_NC = _build_program()
    return _CACHED_NC


def _run(grid_spikes, distance_weights, trace=False):
    grid_spikes = np.ascontiguousarray(np.asarray(grid_spikes, dtype=np.float32))
    distance_weights = np.asarray(distance_weights, dtype=np.float32)
    assert grid_spikes.shape == (B, H, W), grid_spikes.shape
    wb_np = _build_band_weights(distance_weights)

    # pad each sample: 5 wrap rows in front (507..511), 23 behind (0..22)
    xpad = np.concatenate(
        [grid_spikes[:, H - HALO :, :], grid_spikes, grid_spikes[:, :23, :]],
        axis=1,
    )  # [B, 540, W]
    assert xpad.shape[1] == HPAD

    nc = _get_program()
    in_maps = [
        {
            "x": np.ascontiguousarray(xpad[i * BPC : (i + 1) * BPC]),
            "wb": wb_np,
        }
        for i in range(NCORES)
    ]
    res = run_bass_kernel_spmd(nc, in_maps, list(range(NCORES)), trace=trace)
    ypad = np.concatenate(
        [res.results[i]["y"] for i in range(NCORES)], axis=0
    )  # [B, 640, W]
    # unpack: row 103t + p lives at padded row 128t + p (p < 103); the last
    # tile's rows 512..514 are circular duplicates the slice drops.
    out = (
        ypad.reshape(B, NT, 128, W)[:, :, :TR, :]
        .reshape(B, NT * TR, W)[:, :H, :]
    )
    return np.ascontiguousarray(out, dtype=np.float32), res


def kernel(grid_spikes, distance_weights):
    out, _ = _run(grid_spikes, distance_weights, trace=False)
    return out


def kernel_traced(grid_spikes, distance_weights):
    out, res = _run(grid_spikes, distance_weights, trace=True)
    return out, res


# revision 15
# speedup vs baseline: 2.2334x; 1.0059x over previous
"""Trainium2 Bass kernel for the LocalConnectivity diamond-ring stencil.

out[b, x, y] = sum_{1<=|dx|+|dy|<=5} w[|dx|+|dy|-1] * in[b, (x+dx)%512, (y+dy)%512]

Strategy (v6)
-------------
Data-parallel over batch: 64 samples -> 8 cores x 8 samples. Per sample the
512x512 grid is processed in 5 uniform row-tiles of 103 output rows (the 5th
tile computes 3 extra wrapped rows that the host drops), so all 55 matmuls
per sample share identical shapes and the same 11 banded fp16 weight
matrices (input is cast f32->fp16 inside the gpsimd software-DGE DMA).

The 60-tap stencil runs on the TensorEngine as 11 PSUM-accumulating matmuls
per tile, one per horizontal shift dy in [-5, 5], dy-outer so the stationary
matrix is reused across tiles; measured back-to-back at 216 ns/matmul (warm).

THE central hardware fact (microbenchmarked): gpsimd software-DGE DMAs
whose SBUF side spans all 128 partitions run at 300-470 GB/s; any partial
partition range (103/113 rows) takes a degraded ~43 GB/s path on 2 SDMA
engines. So every bulk transfer here is a full-128-partition DMA:

 - INPUT is padded on the host to [8, 540, 512] per core (5 wrap rows in
   front, 23 behind), so each tile-plane loads rows 103t..103t+127 as one
   [128, gsz, 512] DMA with no wrap/halo fixup DMAs at all. 10 input DMAs
   per core, in two sample-groups (2 then 6) to hide startup.
 - OUTPUT goes to a padded y [8, 5*128, 512]: one [128, 5, 512] DMA per
   sample (tile rows 103..127 are garbage the host slices off).
 - Circular COLUMN halos are on-chip ScalarE copies; PSUM eviction is
   split ScalarE (t=0,1) / VectorE (t=2,3,4).
"""

import numpy as np

import concourse.bass as bass
import concourse.bacc as bacc
import concourse.mybir as mybir
from concourse import tile
from concourse.bass_utils import run_bass_kernel_spmd

B, H, W = 64, 512, 512
NCORES = 8
BPC = B // NCORES  # samples per core
MAXD = 5
HALO = MAXD
DYS = 2 * MAXD + 1  # 11 horizontal shifts
TR = 103  # output rows per tile
NT = 5
CTR = TR + 2 * HALO  # 113 contraction rows
XW = W + 2 * HALO  # 522
HPAD = HALO + H + 23  # 540 padded input rows per sample
HOUT = NT * 128  # 640 padded output rows per sample

IN_GROUPS = [(0, 2), (2, 3), (5, 3)]  # (b0, size); staggered to hide wire time

F16 = mybir.dt.float16


def _build_band_weights(dw: np.ndarray) -> np.ndarray:
    """[128, 11*128]: WB[c, j*128 + p] = K(c-p-5, j-5)."""
    wb = np.zeros((128, DYS, 128), dtype=np.float32)
    p = np.arange(128)
    for j in range(DYS):
        dy = j - MAXD
        for dx in range(-MAXD, MAXD + 1):
            d = abs(dx) + abs(dy)
            if 1 <= d <= MAXD:
                c = p + dx + HALO
                valid = (c >= 0) & (c < 128)
                wb[c[valid], j, p[valid]] = dw[d - 1]
    return np.ascontiguousarray(wb.reshape(128, DYS * 128).astype(np.float16))


_CACHED_NC = None


def _custom_ap(base_ap, dims, extra_offset_elems=0):
    """Build a strided AP: dims = [(stride_elems, size), ...]."""
    s = base_ap.copy()
    s.ap.clear()
    s.ap.extend(dims)
    s.offset = s.offset + extra_offset_elems
    return s


def _build_program():
    f32 = mybir.dt.float32

    nc = bacc.Bacc(None, target_bir_lowering=False)
    x = nc.dram_tensor("x", [BPC, HPAD, W], f32, kind="ExternalInput")
    wb = nc.dram_tensor("wb", [128, DYS * 128], F16, kind="ExternalInput")
    y = nc.dram_tensor("y", [BPC, HOUT, W], f32, kind="ExternalOutput")

    with tile.TileContext(nc) as tc:
        with (
            tc.tile_pool(name="wpool", bufs=1) as wpool,
            tc.tile_pool(name="xpool_a", bufs=1) as xpool_a,
            tc.tile_pool(name="xpool_b", bufs=2) as xpool_b,
            tc.tile_pool(name="opool", bufs=3) as opool,
            tc.tile_pool(name="pspool", bufs=8, space=bass.MemorySpace.PSUM) as pspool,
        ):
            wtile = wpool.tile([128, DYS * 128], F16, tag="wt")
            nc.gpsimd.dma_start(wtile[:], wb[:])

            # PE warm-up: ~16 dummy matmuls during the input-load window
            # trip the HAM clock-gate to 8/8 before the real stream starts
            dummy = wpool.tile([128, 640], F16, tag="dummy")
            nc.vector.memset(dummy[:], 0.0)
            wpt = pspool.tile([128, W], mybir.dt.float32, tag="pt")
            for _ in range(16):
                nc.tensor.matmul(wpt[0:TR, :], dummy[0:CTR, 0:TR],
                                 dummy[0:CTR, 64:576], start=True, stop=True)

            sample_xt = {}
            for gi, (pool, (b0, gsz)) in enumerate(
                zip((xpool_a, xpool_b, xpool_b), IN_GROUPS)
            ):
                # xt[p, b, t, 5+y] = xpad[b0+b, 103t + p, y]
                #                  = x_orig[b0+b, (103t - 5 + p) % 512, y]
                xt = pool.tile([128, gsz, NT, XW], F16,
                               tag="xta" if gi == 0 else "xtbc")
                for t in range(NT):
                    src = _custom_ap(
                        x[b0], [(W, 128), (HPAD * W, gsz), (1, W)],
                        extra_offset_elems=TR * t * W,
                    )
                    nc.gpsimd.dma_start(xt[:, :, t, HALO : HALO + W], src)
                    # circular column halos for this plane
                    nc.scalar.copy(
                        xt[0:CTR, :, t, 0:HALO], xt[0:CTR, :, t, W : W + HALO]
                    )
                    nc.scalar.copy(
                        xt[0:CTR, :, t, HALO + W :],
                        xt[0:CTR, :, t, HALO : 2 * HALO],
                    )
                for bi in range(gsz):
                    sample_xt[b0 + bi] = (xt, bi)

            for b in range(BPC):
                xt, bq = sample_xt[b]
                # ---- 55 matmuls: dy-outer, stationary reused over tiles ----
                pts = []
                for t in range(NT):
                    pt = pspool.tile([128, W], f32, tag="pt")
                    pts.append(pt)
                if b == 0:
                    loop = [(j, t) for t in range(NT) for j in range(DYS)]
                else:
                    loop = [(j, t) for j in range(DYS) for t in range(NT)]
                for j, t in loop:
                    nc.tensor.matmul(
                        pts[t][0:TR, :],
                        wtile[0:CTR, j * 128 : j * 128 + TR],
                        xt[0:CTR, bq, t, j : j + W],
                        start=(j == 0),
                        stop=(j == DYS - 1),
                    )

                # ---- PSUM eviction split Scalar/Vector ----
                otb = opool.tile([128, NT, W], f32, tag="otb")
                last = b == BPC - 1
                for t in range(NT):
                    on_scalar = (t < 2) or (last and t == 4)
                    if on_scalar:
                        nc.scalar.copy(otb[0:TR, t, :], pts[t][0:TR, :])
                    else:
                        nc.vector.tensor_copy(otb[0:TR, t, :], pts[t][0:TR, :])

                # ---- full-128-partition output DMAs ----
                if b >= BPC - 2:
                    for lo, hi in ((0, 2), (2, 4), (4, 5)):
                        dstp = _custom_ap(
                            y[b], [(W, 128), (128 * W, hi - lo), (1, W)],
                            extra_offset_elems=lo * 128 * W,
                        )
                        nc.gpsimd.dma_start(dstp, otb[:, lo:hi, :])
                else:
                    dst = _custom_ap(y[b], [(W, 128), (128 * W, NT), (1, W)])
                    nc.gpsimd.dma_start(dst, otb[:, :, :])
    nc.compile()
    return nc


def _get_program():
    global _CACHED_NC
    if _CACHED_NC is None:
        _CACHED_NC = _build_program()
    return _CACHED_NC


def _run(grid_spikes, distance_weights, trace=False):
    grid_spikes = np.ascontiguousarray(np.asarray(grid_spikes, dtype=np.float32))
    distance_weights = np.asarray(distance_weights, dtype=np.float32)
    assert grid_spikes.shape == (B, H, W), grid_spikes.shape
    wb_np = _build_band_weights(distance_weights)

    # pad each sample: 5 wrap rows in front (507..511), 23 behind (0..22)
    xpad = np.concatenate(
        [grid_spikes[:, H - HALO :, :], grid_spikes, grid_spikes[:, :23, :]],
        axis=1,
    )  # [B, 540, W]
    assert xpad.shape[1] == HPAD

    nc = _get_program()
    in_maps = [
        {
            "x": np.ascontiguousarray(xpad[i * BPC : (i + 1) * BPC]),
            "wb": wb_np,
        }
        for i in range(NCORES)
    ]
    res = run_bass_kernel_spmd(nc, in_maps, list(range(NCORES)), trace=trace)
    ypad = np.concatenate(
        [res.results[i]["y"] for i in range(NCORES)], axis=0
    )  # [B, 640, W]
    # unpack: row 103t + p lives at padded row 128t + p (p < 103); the last
    # tile's rows 512..514 are circular duplicates the slice drops.
    out = (
        ypad.reshape(B, NT, 128, W)[:, :, :TR, :]
        .reshape(B, NT * TR, W)[:, :H, :]
    )
    return np.ascontiguousarray(out, dtype=np.float32), res


def kernel(grid_spikes, distance_weights):
    out, _ = _run(grid_spikes, distance_weights, trace=False)
    return out


def kernel_traced(grid_spikes, distance_weights):
    out, res = _run(grid_spikes, distance_weights, trace=True)
    return out, res
